# revision 26
# baseline (speedup 1.0000x reference)
"""Trainium2 Bass kernel for nn_NodeEdgeCrossAttention.

Strategy (dst-sharded, zero-collective, fp16, minimal PE work):
  - Host sorts edges by destination node, greedily assigns nodes to 8 cores
    (balanced edge counts), and packs node edge-runs into 1024-column chunks
    (first-fit-decreasing)
    with a slot pattern shared by all cores (SPMD: one program).  No per-node
    padding: slot boundaries are arbitrary; chunk tails are zero-padded.
  - Scores fold Wq/Wk/scale into per-node M matrices computed on host (O(N)):
    score[e,h] = M[dst_e,h] . k_raw_e.  bk cancels by softmax shift
    invariance; bv folds through Wo into the bias since sum(attn)==1.
  - Wv folds into Wo per head (W2_h = Wv[:,h] @ Wo[h,:]), so the device
    aggregates RAW v vectors; no per-edge v projection on device.
  - Per chunk on device: 1 fused DMA; NT score matmuls (k-tile
    stationary, M moving) -> [c=128, 4ns] PSUM per tile; 1 exp (scalar
    engine); 1 mask-multiply with the one-hot S (vector engine) -> exm;

    NT segment matmuls (exm stationary, [v|1] moving) accumulating
    [4ns, 129] in PSUM (weighted-v sums + softmax denominators);
    reciprocal of den + normalize multiply (vector) -> fp16; PE transpose
    into a persistent SBUF staging buffer.
  - Output blocks of 512 slots are projected with host-folded W2 + bias as
    soon as their slots are staged (overlapped with the chunk stream).
  - Numerics: fp16 linear tensors (fp8 fails the 2e-2 gate: attention-weight
    quantization error does not average down relative to the output), fp32
    accumulation, exp emits fp16 with a -ln16 bias that cancels in the
    normalize.
"""

import numpy as np

N, E, DIM, HEADS = 10000, 640000, 128, 4
DH = DIM // HEADS
NCORES = 8
CHUNK = 1024
NT = CHUNK // 128                 # k/v tiles per chunk
TILE = 128
SCALE = DH ** -0.5
NSMAX = 32                      # slots per chunk cap (PSUM: 4*NT*ns <= 1024 fp32)
LN16 = float(np.log(16.0))


class Plan:
    pass


def _make_plan(dst):
    """Greedy core assignment + shared chunk/slot pattern (no per-node pad)."""
    deg = np.bincount(dst, minlength=N)
    nz = np.where(deg > 0)[0]
    if deg.max() > CHUNK:
        raise NotImplementedError(f"max degree {deg.max()} > {CHUNK}")

    order = nz[np.argsort(-deg[nz], kind="stable")]
    loads = np.zeros(NCORES, np.int64)
    core_nodes = [[] for _ in range(NCORES)]
    for n in order:
        c = int(loads.argmin())
        core_nodes[c].append(int(n))
        loads[c] += deg[n]

    # Shared slot pattern: rank r -> max deg across cores at that rank.
    L = max(len(cn) for cn in core_nodes)
    pat = np.zeros(L, np.int64)
    for cn in core_nodes:
        d = deg[np.array(cn, np.int64)]
        pat[: len(d)] = np.maximum(pat[: len(d)], d)

    # First-fit decreasing bin packing of pattern slots into 512-col chunks.
    chunks = []                 # list of dict(slots=[(rank, col0, R)])
    open_rem = []               # remaining cols per open chunk
    for r in range(L):
        R = int(pat[r])
        placed = False
        for ci in range(len(chunks)):
            if open_rem[ci] >= R and len(chunks[ci]["slots"]) < NSMAX:
                col0 = CHUNK - open_rem[ci]
                chunks[ci]["slots"].append((r, col0, R))
                open_rem[ci] -= R
                placed = True
                break
        if not placed:
            chunks.append({"slots": [(r, 0, R)]})
            open_rem.append(CHUNK - R)

    # Global slot index g in (chunk, slot) order; map rank -> g.
    rank2g = np.full(L, -1, np.int64)
    g = 0
    for ch in chunks:
        ch["g0"] = g
        ch["ns"] = len(ch["slots"])
        for (r, _, _) in ch["slots"]:
            if r >= 0:
                rank2g[r] = g
            g += 1

    p = Plan()
    p.deg = deg
    p.core_nodes = core_nodes
    p.chunks = chunks
    p.nchunks = len(chunks)
    p.L = L
    p.G = g                                   # total slots
    p.SLOTP = ((g + 511) // 512) * 512
    p.rank2g = rank2g
    # chunk fp16-element widths in the fused kvs tensor:
    # K(CHUNK) | V(NT*129) | S(NT*ns) | M(4ns)
    p.offs = []
    off = 0
    for ch in chunks:
        ns = ch["ns"]
        w = CHUNK + 129 * NT + (NT + 4) * ns
        p.offs.append(off)
        off += w
    p.KVW_total = off
    return p


def _pack_core_inputs(plan, c, kT_ext, v_ext, Mfull, edges_of):
    """Per-core fused kvs [128, KVW_total] fp16."""
    deg = plan.deg
    cn = plan.core_nodes[c]
    ncols = plan.nchunks * CHUNK
    idx = np.full(ncols, E, np.int64)          # E -> zero sentinel column
    for ch_i, ch in enumerate(plan.chunks):
        for (r, col0, R) in ch["slots"]:
            if r < 0 or r >= len(cn):
                continue
            node = cn[r]
            d = deg[node]
            g0 = ch_i * CHUNK + col0
            idx[g0: g0 + d] = edges_of[node]

    kT = kT_ext[:, idx]                        # [128, ncols] fp16
    vE = v_ext[idx]                            # [ncols, 128] fp16

    kvs = np.zeros((TILE, plan.KVW_total), np.float16)
    for ch_i, ch in enumerate(plan.chunks):
        ns = ch["ns"]
        off = plan.offs[ch_i]
        c0 = ch_i * CHUNK
        # K section [128 d, CHUNK c]
        kvs[:, off: off + CHUNK] = kT[:, c0: c0 + CHUNK]
        # V section: NT x [128 c, 128 d | 1]
        voff = off + CHUNK
        for t in range(NT):
            blk = kvs[:, voff + 129 * t: voff + 129 * t + 129]
            blk[:, 0:128] = vE[c0 + 128 * t: c0 + 128 * (t + 1)]
            blk[:, 128] = 1.0
        # S section: one-hot [128 c, (t, j)]
        soff = voff + 129 * NT
        for j, (r, col0, R) in enumerate(ch["slots"]):
            if r < 0 or r >= len(cn):
                continue
            d = int(deg[cn[r]])
            for t in range(NT):
                lo = max(col0, t * TILE)
                hi = min(col0 + d, (t + 1) * TILE)
                if lo < hi:
                    kvs[lo - t * TILE: hi - t * TILE, soff + t * ns + j] = 1.0
        # M section: [128 d, (j, h)]
        moff = soff + NT * ns
        for j, (r, col0, R) in enumerate(ch["slots"]):
            if 0 <= r < len(cn):
                kvs[:, moff + 4 * j: moff + 4 * j + 4] = Mfull[:, cn[r], :]
    return kvs


# ---------------------------------------------------------------------------
# Device kernel emission
# ---------------------------------------------------------------------------

def _build_module(plan):
    import concourse.bacc as bacc
    import concourse.mybir as mybir
    import concourse.tile as tile
    from contextlib import ExitStack

    f16 = mybir.dt.float16
    f32 = mybir.dt.float32
    SLOTP = plan.SLOTP
    NBLK = SLOTP // 512

    nc = bacc.Bacc("TRN2", debug=False, num_devices=NCORES)

    kvs_d = nc.dram_tensor("kvs", [TILE, plan.KVW_total], f16,
                           kind="ExternalInput")
    W2_d = nc.dram_tensor("W2", [DIM, 4 * DIM], f16, kind="ExternalInput")
    bo2_d = nc.dram_tensor("bo2", [DIM, 1], f32, kind="ExternalInput")
    ID_d = nc.dram_tensor("ID", [DIM, DIM], f16, kind="ExternalInput")
    outT_d = nc.dram_tensor("outT", [DIM, SLOTP], f32, kind="ExternalOutput")

    Exp = mybir.ActivationFunctionType.Exp
    Ident = mybir.ActivationFunctionType.Identity
    mult = mybir.AluOpType.mult
    amax = mybir.AluOpType.max

    with ExitStack() as ctx:
        tc = ctx.enter_context(tile.TileContext(nc))
        cp = ctx.enter_context(tc.tile_pool(name="const", bufs=1))
        sp = ctx.enter_context(tc.tile_pool(name="persist", bufs=1))
        iop = ctx.enter_context(tc.tile_pool(name="io", bufs=12))
        xp = ctx.enter_context(tc.tile_pool(name="work", bufs=4))
        pp = ctx.enter_context(tc.tile_pool(name="ps", bufs=2, space="PSUM"))
        pp1 = ctx.enter_context(tc.tile_pool(name="ps1", bufs=1, space="PSUM"))
        dp = ctx.enter_context(tc.tile_pool(name="dummy", bufs=1, space="PSUM"))

        W2_sb = cp.tile([DIM, 4 * DIM], f16)
        nc.sync.dma_start(out=W2_sb[:], in_=W2_d[:, :])
        bo2_sb = cp.tile([DIM, 1], f32)
        nc.sync.dma_start(out=bo2_sb[:], in_=bo2_d[:, :])
        ln16_sb = cp.tile([DIM, 1], f32)
        nc.gpsimd.memset(ln16_sb[:], -LN16)
        ID_sb = cp.tile([DIM, DIM], f16)
        nc.sync.dma_start(out=ID_sb[:], in_=ID_d[:, :])
        stag = sp.tile([TILE, 4 * SLOTP], f16)
        stag_r = stag[:].rearrange("p (s h) -> p s h", h=4)

        def emit_block(b):
            out_ps = pp.tile([TILE, NT * TILE], f32, tag="score")
            for h in range(4):
                nc.tensor.matmul(
                    out=out_ps[:, 0:512],
                    lhsT=W2_sb[:, h * DIM:(h + 1) * DIM],
                    rhs=stag_r[:, b * 512:(b + 1) * 512, h],
                    start=(h == 0), stop=(h == 3))
            osb = xp.tile([DIM, 512], f32, tag="osb")
            nc.scalar.activation(out=osb[:], in_=out_ps[:, 0:512],
                                 func=Ident, bias=bo2_sb[:, 0:1])
            nc.sync.dma_start(out=outT_d[:, b * 512:(b + 1) * 512],
                              in_=osb[:])

        next_block = 0

        VOFF = CHUNK
        SOFF = CHUNK + 129 * NT

        kvts = {}

        def dma_chunk(j):
            if j >= plan.nchunks:
                return
            ch = plan.chunks[j]
            W = SOFF + (NT + 4) * ch["ns"]
            kvt = iop.tile([TILE, SOFF + (NT + 4) * NSMAX], f16, tag="kv")
            nc.sync.dma_start(out=kvt[:, 0:W],
                              in_=kvs_d[:, plan.offs[j]: plan.offs[j] + W])
            kvts[j] = kvt

        st = {}                  # per-chunk in-flight tiles
        PF = 6                   # DMA prefetch distance
        for j in range(PF):
            dma_chunk(j)

        # HAM warmup + keep-warm fillers (PE at 2.4 GHz is worth ~5us here)
        warm_ps = dp.tile([1, TILE], f32, tag="warm")

        def keep_warm(n, cols=TILE):
            for _ in range(n):
                nc.tensor.matmul(out=warm_ps[0:1, 0:cols],
                                 lhsT=W2_sb[:, 0:1],
                                 rhs=W2_sb[:, 0:cols],
                                 start=True, stop=True)

        keep_warm(40)


        # Software-pipelined steady state with a 4-deep skew so that every
        # op is data-ready when its engine reaches it (strict-FIFO queues):
        #   PE:  scores_i | seg_{i-2} | transpose_{i-4}
        #   ACT: exp_i | stage-copy_{i-4}
        #   DVE: mask_{i-1} | rcp_{i-3} | normalize_{i-3}
        for i in range(plan.nchunks + 4):
            if i < plan.nchunks:
                ch = plan.chunks[i]
                ns = ch["ns"]
                kvt = kvts[i]
                ksec = kvt[:, 0:CHUNK]
                Msec = kvt[:, SOFF + NT * ns: SOFF + (NT + 4) * ns]
                score_ps = pp.tile([TILE, NT * TILE], f32, tag="score")
                for t in range(NT):
                    nc.tensor.matmul(
                        out=score_ps[:, t * TILE: t * TILE + 4 * ns],
                        lhsT=ksec[:, t * TILE:(t + 1) * TILE],
                        rhs=Msec[:],
                        start=True, stop=True)
                st[i] = {"ns": ns, "score": score_ps, "ch": ch}

            if 0 <= i - 2 < plan.nchunks:
                keep_warm(6, 64)
                s = st[i - 2]
                ns = s["ns"]
                kvt = kvts[i - 2]
                vsec = kvt[:, VOFF:SOFF]
                park = pp.tile([TILE, 129], f32, tag="park")
                for t in range(NT):
                    nc.tensor.matmul(
                        out=park[0:4 * ns, :],
                        lhsT=s["exm"][:, t * TILE: t * TILE + 4 * ns],
                        rhs=vsec[:, 129 * t: 129 * t + 129],
                        start=(t == 0), stop=(t == NT - 1))
                s["park"] = park


            if 0 <= i - 4 < plan.nchunks:
                s = st[i - 4]
                ns = s["ns"]
                tp_ps = pp1.tile([TILE, TILE], f16, tag="tp")
                nc.tensor.transpose(out=tp_ps[0:TILE, 0:4 * ns],
                                    in_=s["aggN"][0:4 * ns, :],
                                    identity=ID_sb[0:4 * ns, 0:4 * ns])
                s["tp"] = tp_ps
                keep_warm(6, 64)

            if i < plan.nchunks:
                s = st[i]
                ns = s["ns"]
                exs = xp.tile([TILE, NT * TILE], f16, tag="exs")
                score_r = s["score"][:].rearrange("p (t c) -> p t c", t=NT)
                exs_r = exs[:].rearrange("p (t c) -> p t c", t=NT)
                nc.scalar.activation(out=exs_r[:, :, 0:4 * ns],
                                     in_=score_r[:, :, 0:4 * ns],
                                     func=Exp, bias=ln16_sb[:, 0:1])
                s["exs"] = exs

            if 0 <= i - 4 < plan.nchunks:
                s = st[i - 4]
                ns = s["ns"]
                g0 = s["ch"]["g0"]
                nc.scalar.copy(out=stag[:, 4 * g0: 4 * (g0 + ns)],
                               in_=s["tp"][0:TILE, 0:4 * ns])

            if 0 <= i - 1 < plan.nchunks:
                s = st[i - 1]
                ns = s["ns"]
                kvt = kvts[i - 1]
                Ssec = kvt[:, SOFF:SOFF + NT * ns]
                exm = xp.tile([TILE, NT * TILE], f16, tag="exm")
                exm_r = exm[:].rearrange("p (t c) -> p t c", t=NT)
                exs_r = s["exs"][:].rearrange("p (t c) -> p t c", t=NT)
                nc.vector.tensor_tensor(
                    out=exm_r[:, :, 0:4 * ns].rearrange(
                        "p t (j h) -> p t j h", h=4),
                    in0=exs_r[:, :, 0:4 * ns].rearrange(
                        "p t (j h) -> p t j h", h=4),
                    in1=Ssec[:].rearrange("p (t j) -> p t j", t=NT)
                        .unsqueeze(-1).to_broadcast([TILE, NT, ns, 4]),
                    op=mult)
                s["exm"] = exm

            if 0 <= i - 3 < plan.nchunks:
                s = st[i - 3]
                ns = s["ns"]
                park = s["park"]
                rdent = xp.tile([TILE, 1], f32, tag="rden")
                nc.vector.reciprocal(out=rdent[0:4 * ns, 0:1],
                                     in_=park[0:4 * ns, 128:129])
                aggN = xp.tile([TILE, TILE], f16, tag="aggN")
                nc.vector.tensor_tensor(
                    out=aggN[0:4 * ns, :],
                    in0=park[0:4 * ns, 0:128],
                    in1=rdent[0:4 * ns, 0:1].to_broadcast([4 * ns, TILE]),
                    op=mult)
                s["aggN"] = aggN

            if i >= 4:
                st.pop(i - 4, None)
            dma_chunk(i + PF)

        while next_block < NBLK:
            emit_block(next_block)
            next_block += 1

    nc.compile()
    return nc


# ---------------------------------------------------------------------------
# Entry point
# ---------------------------------------------------------------------------

def _prepare(inputs):
    q_nodes = np.asarray(inputs["q_nodes"], np.float32)
    k_edges = np.asarray(inputs["k_edges"], np.float32)
    v_edges = np.asarray(inputs["v_edges"], np.float32)
    Wq = np.asarray(inputs["Wq"], np.float32)
    bq = np.asarray(inputs["bq"], np.float32)
    Wk = np.asarray(inputs["Wk"], np.float32)
    Wv = np.asarray(inputs["Wv"], np.float32)
    bv = np.asarray(inputs["bv"], np.float32)
    Wo = np.asarray(inputs["Wo"], np.float32)
    bo = np.asarray(inputs["bo"], np.float32)
    dst = np.asarray(inputs["edge_index"])[0].astype(np.int64)

    plan = _make_plan(dst)

    eorder = np.argsort(dst, kind="stable")
    starts = np.zeros(N + 1, np.int64)
    np.cumsum(np.bincount(dst, minlength=N), out=starts[1:])
    edges_of = [eorder[starts[n]: starts[n + 1]] for n in range(N)]

    # host-side per-node score matrices M[d, n, h] and folded weights
    qp = q_nodes @ Wq + bq
    Mfull = np.empty((DIM, N, HEADS), np.float32)
    for h in range(HEADS):
        sl = slice(h * DH, (h + 1) * DH)
        Mfull[:, :, h] = (Wk[:, sl] * SCALE) @ qp[:, sl].T
    Mfull = Mfull.astype(np.float16)

    W2 = np.empty((DIM, 4 * DIM), np.float32)
    for h in range(HEADS):
        sl = slice(h * DH, (h + 1) * DH)
        W2[:, h * DIM:(h + 1) * DIM] = Wv[:, sl] @ Wo[sl, :]
    consts = {
        "W2": W2.astype(np.float16),
        "bo2": (bv @ Wo + bo).reshape(DIM, 1).astype(np.float32),
        "ID": np.eye(DIM, dtype=np.float16),
    }

    kT_ext = np.zeros((DIM, E + 1), np.float16)
    kT_ext[:, :E] = k_edges.T.astype(np.float16)
    v_ext = np.zeros((E + 1, DIM), np.float16)
    v_ext[:E] = v_edges.astype(np.float16)

    return plan, edges_of, consts, kT_ext, v_ext, Mfull, bo


def kernel(**inputs):
    from concourse.bass_utils import run_bass_kernel_spmd

    plan, edges_of, consts, kT_ext, v_ext, Mfull, bo = _prepare(inputs)

    nc = _build_module(plan)

    in_maps = []
    for c in range(NCORES):
        kvs = _pack_core_inputs(plan, c, kT_ext, v_ext, Mfull, edges_of)
        m = {"kvs": kvs}
        m.update(consts)
        in_maps.append(m)

    res = run_bass_kernel_spmd(nc, in_maps, core_ids=list(range(NCORES)))
    global LAST_RESULTS
    LAST_RESULTS = res

    out = np.zeros((N, DIM), np.float32)
    for c in range(NCORES):
        outT = res.results[c]["outT"]              # [128, SLOTP]
        cn = plan.core_nodes[c]
        gs = plan.rank2g[: len(cn)]
        out[np.array(cn, np.int64)] = outT[:, gs].T
    deg0 = plan.deg == 0
    if deg0.any():
        out[deg0] = bo
    return out
